# revision 8
# baseline (speedup 1.0000x reference)
"""Bass/Trainium2 kernel for nn_GRUSingleRollout.

2-layer bidirectional masked GRU encoders (agents + ego) + fuse MLP + head,
data-parallel over 8 NeuronCores on the flattened agent dim (1024 agents/core),
ego encoder replicated per-core (host-rotated so the SPMD program is
core-id-independent).

Layout on device: gates on partitions, sequences on the free dim.  fwd dir in
partitions 0-63, bwd dir in partitions 64-127 of every tile.  The bwd scan is
fed the time-reversed input stream (prepared host-side), so both directions
step s = 0..63 together.  Masking uses z-gate saturation: +30*(1-m) is added to
the z pre-activation via dedicated matmul input rows, so sigmoid(z)==1 and h is
frozen exactly on masked steps (replicating jnp.where(m, hn, h)).

All matmuls bf16 (1 cyc/col on PE), accumulation fp32 in PSUM; sigmoid/tanh on
ACT (one table set serves both); GRU cell elementwise on DVE with fused
scalar_tensor_tensor ops; h / layer-0 outputs stored bf16 in SBUF.
"""

import sys
from contextlib import ExitStack

sys.path.insert(0, "/opt/trn_rl_repo")

import numpy as np
import ml_dtypes

import concourse.bass as bass
import concourse.tile as tile
from concourse import bacc
from concourse import mybir

F32 = mybir.dt.float32
BF16 = mybir.dt.bfloat16
BF = ml_dtypes.bfloat16
AF = mybir.ActivationFunctionType
OP = mybir.AluOpType

B, NA, T, TF = 128, 64, 64, 30
H = 64          # per-direction hidden
D_MODEL = 128
NCORES = 8
BS = B // NCORES            # 16 batches per core
SAG = BS * NA               # 1024 agent seqs per core
NC_AG = 512                 # agent chunk width
NCHUNK = SAG // NC_AG       # 2
NC_EGO = B                  # 128 ego seqs (full, replicated)
MASKW = 30.0


# ---------------------------------------------------------------- host prep

def _xstack(x, m):
    """x (N,T,2) f32, m (N,T) bool -> (6, T*N) bf16.

    rows: 0 maskinv fwd, 1 maskinv rev, 2-3 x fwd, 4-5 x rev.  col t*N+j."""
    N = x.shape[0]
    xs = np.zeros((6, T, N), np.float32)
    mi = MASKW * (1.0 - m.astype(np.float32).T)     # (T,N)
    xs[0] = mi
    xs[1] = mi[::-1]
    xs[2] = x[:, :, 0].T
    xs[3] = x[:, :, 1].T
    xs[4] = xs[2][::-1]
    xs[5] = xs[3][::-1]
    return xs.reshape(6, T * N).astype(BF)


def _prep_weights(params):
    """params: ((dir0, dir1) per layer), dir = (Wih(192,d), Whh(192,64), bih, bhh).
    Returns dict of host arrays."""
    out = {}
    (f0, b0), (f1, b1) = params[0], params[1]
    for g, gn in enumerate("rzn"):
        sl = slice(g * H, (g + 1) * H)
        # L0 gi lhsT (6,128): rows 0/1 mask (z only), 2-3 Wih_f, 4-5 Wih_b
        l = np.zeros((6, 128), np.float32)
        if gn == "z":
            l[0, :H] = 1.0
            l[1, H:] = 1.0
        l[2, :H] = f0[0][sl, 0]
        l[3, :H] = f0[0][sl, 1]
        l[4, H:] = b0[0][sl, 0]
        l[5, H:] = b0[0][sl, 1]
        out[f"gi0_{gn}"] = l.astype(BF)
        # L1 gi lhsT (128,64) per dir: Wih1_dir_g.T
        out[f"gi1f_{gn}"] = np.ascontiguousarray(f1[0][sl].T).astype(BF)
        out[f"gi1b_{gn}"] = np.ascontiguousarray(b1[0][sl].T).astype(BF)
        # rec lhsT (128,64): rows 0-63 Whh_f_g.T, 64-127 Whh_b_g.T
        for li, (fd, bd) in enumerate(((f0, b0), (f1, b1))):
            r = np.zeros((128, H), np.float32)
            r[:H] = fd[1][sl].T
            r[H:] = bd[1][sl].T
            out[f"rec{li}_{gn}"] = r.astype(BF)
    for li, (fd, bd) in enumerate(((f0, b0), (f1, b1))):
        br = np.concatenate([fd[2][0:H] + fd[3][0:H], bd[2][0:H] + bd[3][0:H]])
        bz = np.concatenate([fd[2][H:2*H] + fd[3][H:2*H], bd[2][H:2*H] + bd[3][H:2*H]])
        bni = np.concatenate([fd[2][2*H:], bd[2][2*H:]])
        bnh = np.concatenate([fd[3][2*H:], bd[3][2*H:]])
        out[f"bias{li}_r"] = br.reshape(128, 1).astype(np.float32)
        out[f"bias{li}_z"] = bz.reshape(128, 1).astype(np.float32)
        out[f"bias{li}_ni"] = bni.reshape(128, 1).astype(np.float32)
        out[f"bias{li}_nh"] = bnh.reshape(128, 1).astype(np.float32)
    return out


# ---------------------------------------------------------------- device program

def build_nc():
    nc = bacc.Bacc()
    TN_AG = T * NC_AG
    TN_EG = T * NC_EGO

    # dram params
    xst = [nc.declare_dram_parameter(f"xst{n}", [6, TN_AG], BF16, isOutput=False) for n in range(NCHUNK)]
    xste = nc.declare_dram_parameter("xste", [6, TN_EG], BF16, isOutput=False)
    wnames = []
    for pfx in ("ag", "eg"):
        for g in "rzn":
            wnames += [f"{pfx}_gi0_{g}", f"{pfx}_gi1f_{g}", f"{pfx}_gi1b_{g}",
                       f"{pfx}_rec0_{g}", f"{pfx}_rec1_{g}"]
    wshapes = {"gi0": [6, 128], "gi1f": [128, H], "gi1b": [128, H],
               "rec0": [128, H], "rec1": [128, H]}
    wdr = {}
    for n in wnames:
        kind = n.split("_")[1]
        wdr[n] = nc.declare_dram_parameter(n, wshapes[kind], BF16, isOutput=False)
    bdr = {}
    for pfx in ("ag", "eg"):
        for li in range(2):
            for b in ("r", "z", "ni", "nh"):
                n = f"{pfx}_bias{li}_{b}"
                bdr[n] = nc.declare_dram_parameter(n, [128, 1], F32, isOutput=False)
    mask1 = nc.declare_dram_parameter("mask1", [2, 128], BF16, isOutput=False)
    w1a = nc.declare_dram_parameter("w1a", [128, 128], BF16, isOutput=False)
    w1e = nc.declare_dram_parameter("w1e", [128, 128], BF16, isOutput=False)
    w2t = nc.declare_dram_parameter("w2t", [128, 128], BF16, isOutput=False)
    w3t = nc.declare_dram_parameter("w3t", [128, 2 * TF], BF16, isOutput=False)
    b1d = nc.declare_dram_parameter("b1", [128, 1], F32, isOutput=False)
    b2d = nc.declare_dram_parameter("b2", [128, 1], F32, isOutput=False)
    b3d = nc.declare_dram_parameter("b3", [2 * TF, 1], F32, isOutput=False)
    outd = nc.declare_dram_parameter("out", [2 * TF, SAG], F32, isOutput=True)

    with tile.TileContext(nc) as tc, ExitStack() as ctx:
        wpool = ctx.enter_context(tc.tile_pool(name="w", bufs=1))
        big = ctx.enter_context(tc.tile_pool(name="big", bufs=1))
        hpool = ctx.enter_context(tc.tile_pool(name="h", bufs=1))
        ew = ctx.enter_context(tc.tile_pool(name="ew", bufs=3))
        psum = ctx.enter_context(tc.tile_pool(name="ps", bufs=2, space=bass.MemorySpace.PSUM))

        # --- load weights
        wt = {}
        for n, d in {**wdr, **bdr}.items():
            dt = BF16 if n in wdr else F32
            s = d.shape
            wtile = wpool.tile(list(s), dt, tag=n)
            nc.gpsimd.dma_start(wtile[:], d[:])
            wt[n] = wtile
        for n, d in (("mask1", mask1), ("w1a", w1a), ("w1e", w1e), ("w2t", w2t),
                     ("w3t", w3t), ("b1", b1d), ("b2", b2d), ("b3", b3d)):
            dt = F32 if n in ("b1", "b2", "b3") else BF16
            wtile = wpool.tile(list(d.shape), dt, tag=n)
            nc.gpsimd.dma_start(wtile[:], d[:])
            wt[n] = wtile

        def scan(pfx, xdram, Nc, hkey):
            """Run the 2-layer bidirectional GRU on one chunk.
            Returns hL1 tile (128,Nc) = [hf1; hb1]."""
            TN = T * Nc
            xt = big.tile([6, TN], BF16, tag="xst")
            nc.gpsimd.dma_start(xt[:], xdram[:])
            o0 = big.tile([128, TN], BF16, tag="o0")
            h0 = hpool.tile([128, Nc], BF16, tag=f"{hkey}_h0")
            h1 = hpool.tile([128, Nc], BF16, tag=f"{hkey}_h1")

            for li in range(2):
                for s in range(T):
                    blk = bass.ts(s, Nc)
                    rblk = bass.ts(T - 1 - s, Nc)
                    RR = psum.tile([128, Nc], F32, tag="RR")
                    ZZ = psum.tile([128, Nc], F32, tag="ZZ")
                    GHN = psum.tile([128, Nc], F32, tag="GHN")
                    GIN = psum.tile([128, Nc], F32, tag="GIN")
                    first = s == 0
                    # input transforms
                    if li == 0:
                        for g, ps in (("r", RR), ("z", ZZ), ("n", GIN)):
                            nc.tensor.matmul(ps[:], wt[f"{pfx}_gi0_{g}"][:],
                                             xt[:, blk], start=True, stop=first and g != "n",
                                             skip_group_check=True)
                    else:
                        for g, ps in (("r", RR), ("z", ZZ), ("n", GIN)):
                            nc.tensor.matmul(ps[0:H, :], wt[f"{pfx}_gi1f_{g}"][:],
                                             o0[:, blk], start=True, stop=False,
                                             skip_group_check=True)
                            nc.tensor.matmul(ps[H:128, :], wt[f"{pfx}_gi1b_{g}"][:],
                                             o0[:, rblk], start=True,
                                             stop=(first and g == "r") or g == "n",
                                             skip_group_check=True)
                        nc.tensor.matmul(ZZ[:], wt["mask1"][:], xt[0:2, blk],
                                         start=False, stop=first, skip_group_check=True)
                    if not first:
                        # recurrent parts; fwd from partitions 0-63, bwd 64-127
                        if li == 0:
                            hf = o0[0:H, bass.ts(s - 1, Nc)]
                            hb = o0[H:128, bass.ts(T - s, Nc)]
                        else:
                            hp = h0 if (s - 1) % 2 == 0 else h1
                            hf, hb = hp[0:H, :], hp[H:128, :]
                        for g, ps in (("r", RR), ("z", ZZ), ("n", GHN)):
                            w = wt[f"{pfx}_rec{li}_{g}"]
                            nc.tensor.matmul(ps[0:H, :], w[0:H, :], hf,
                                             start=g == "n", stop=True,
                                             skip_group_check=True)
                            nc.tensor.matmul(ps[H:128, :], w[H:128, :], hb,
                                             start=g == "n", stop=True,
                                             skip_group_check=True)
                    # elementwise GRU cell
                    Rs = ew.tile([128, Nc], BF16, tag="Rs")
                    Zs = ew.tile([128, Nc], BF16, tag="Zs")
                    t2 = ew.tile([128, Nc], BF16, tag="t2")
                    nn_ = ew.tile([128, Nc], BF16, tag="nn")
                    dd = ew.tile([128, Nc], BF16, tag="dd")
                    ee = ew.tile([128, Nc], BF16, tag="ee")
                    bias = lambda b: wt[f"{pfx}_bias{li}_{b}"][:]
                    nc.scalar.activation(Rs[:], RR[:], AF.Sigmoid, bias=bias("r"))
                    nc.scalar.activation(Zs[:], ZZ[:], AF.Sigmoid, bias=bias("z"))
                    if first:
                        # h=0: n = tanh(gin + bini + r*bnh); h' = n - z*n
                        t1 = ew.tile([128, Nc], BF16, tag="t1")
                        nc.vector.tensor_scalar(t1[:], Rs[:], bias("nh"), None, OP.mult)
                        nc.vector.tensor_tensor(t2[:], t1[:], GIN[:], OP.add)
                        nc.scalar.activation(nn_[:], t2[:], AF.Tanh, bias=bias("ni"))
                        nc.vector.tensor_tensor(ee[:], Zs[:], nn_[:], OP.mult)
                        if li == 0:
                            nc.vector.scalar_tensor_tensor(
                                o0[0:H, blk], ee[0:H, :], -1.0, nn_[0:H, :],
                                OP.mult, OP.add)
                            nc.vector.scalar_tensor_tensor(
                                o0[H:128, rblk], ee[H:128, :], -1.0, nn_[H:128, :],
                                OP.mult, OP.add)
                        else:
                            hn = h0 if s % 2 == 0 else h1
                            nc.vector.scalar_tensor_tensor(
                                hn[:], ee[:], -1.0, nn_[:], OP.mult, OP.add)
                    else:
                        # t2 = (GHN + bnh) * Rs + GIN ; n = tanh(t2 + bni)
                        t1 = ew.tile([128, Nc], BF16, tag="t1")
                        nc.vector.scalar_tensor_tensor(t1[:], GHN[:], bias("nh"),
                                                       Rs[:], OP.add, OP.mult)
                        nc.vector.tensor_tensor(t2[:], t1[:], GIN[:], OP.add)
                        nc.scalar.activation(nn_[:], t2[:], AF.Tanh, bias=bias("ni"))
                        # d = h - n ; e = z*d ; h' = n + e
                        if li == 0:
                            nc.vector.scalar_tensor_tensor(
                                dd[0:H, :], nn_[0:H, :], -1.0,
                                o0[0:H, bass.ts(s - 1, Nc)], OP.mult, OP.add)
                            nc.vector.scalar_tensor_tensor(
                                dd[H:128, :], nn_[H:128, :], -1.0,
                                o0[H:128, bass.ts(T - s, Nc)], OP.mult, OP.add)
                            nc.vector.tensor_tensor(ee[:], Zs[:], dd[:], OP.mult)
                            nc.vector.tensor_tensor(o0[0:H, blk], ee[0:H, :],
                                                    nn_[0:H, :], OP.add)
                            nc.vector.tensor_tensor(o0[H:128, rblk], ee[H:128, :],
                                                    nn_[H:128, :], OP.add)
                        else:
                            hp = h0 if (s - 1) % 2 == 0 else h1
                            hn = h0 if s % 2 == 0 else h1
                            nc.vector.scalar_tensor_tensor(
                                dd[:], nn_[:], -1.0, hp[:], OP.mult, OP.add)
                            nc.vector.tensor_tensor(ee[:], Zs[:], dd[:], OP.mult)
                            nc.vector.tensor_tensor(hn[:], ee[:], nn_[:], OP.add)
            return h1  # T-1 = 63 -> parity 1

        hego = scan("eg", xste, NC_EGO, "eg")
        hag = [scan("ag", xst[n], NC_AG, f"c{n}") for n in range(NCHUNK)]

        # --- fuse + head
        # ego contribution: eb[:,b] = W1e @ hego[:,b] + b1   (fp32, per-batch bias)
        egz = psum.tile([128, NC_EGO], F32, tag="RR")
        nc.tensor.matmul(egz[:], wt["w1e"][:], hego[:], start=True, stop=True,
                         skip_group_check=True)
        eb = wpool.tile([128, NC_EGO], F32, tag="eb")
        nc.scalar.activation(eb[:], egz[:], AF.Identity, bias=wt["b1"][:])
        for n in range(NCHUNK):
            zp1 = psum.tile([128, NC_AG], F32, tag="ZZ")
            nc.tensor.matmul(zp1[:], wt["w1a"][:], hag[n][:], start=True, stop=True,
                             skip_group_check=True)
            z1 = ew.tile([128, NC_AG], BF16, tag="z1")
            for g in range(NC_AG // NA):
                bcol = n * (NC_AG // NA) + g
                nc.scalar.activation(z1[:, bass.ts(g, NA)], zp1[:, bass.ts(g, NA)],
                                     AF.Relu, bias=eb[:, bcol:bcol + 1])
            zp2 = psum.tile([128, NC_AG], F32, tag="GHN")
            nc.tensor.matmul(zp2[:], wt["w2t"][:], z1[:], start=True, stop=True,
                             skip_group_check=True)
            z2 = ew.tile([128, NC_AG], BF16, tag="z2")
            nc.scalar.activation(z2[:], zp2[:], AF.Relu, bias=wt["b2"][:])
            op3 = psum.tile([2 * TF, NC_AG], F32, tag="GIN")
            nc.tensor.matmul(op3[:], wt["w3t"][:], z2[:], start=True, stop=True,
                             skip_group_check=True)
            o3 = ew.tile([2 * TF, NC_AG], F32, tag="o3")
            nc.scalar.activation(o3[:], op3[:], AF.Identity, bias=wt["b3"][:])
            nc.gpsimd.dma_start(outd[:, bass.ts(n, NC_AG)], o3[:])
    nc.compile()
    return nc


# ---------------------------------------------------------------- entry point

def kernel(agents_hist_xy, agents_hist_mask, ego_hist_xy, ego_hist_mask,
           ag_params, ego_params, fuse_W1, fuse_b1, fuse_W2, fuse_b2,
           head_W, head_b):
    tonp = lambda a: np.asarray(a)
    x_ag = tonp(agents_hist_xy).reshape(B * NA, T, 2)
    m_ag = tonp(agents_hist_mask).reshape(B * NA, T)
    x_eg = tonp(ego_hist_xy)
    m_eg = tonp(ego_hist_mask)
    agp = tuple(tuple(tuple(tonp(w) for w in d) for d in l) for l in ag_params)
    egp = tuple(tuple(tuple(tonp(w) for w in d) for d in l) for l in ego_params)

    wag = _prep_weights(agp)
    weg = _prep_weights(egp)
    mask1 = np.zeros((2, 128), np.float32)
    mask1[0, :H] = 1.0
    mask1[1, H:] = 1.0
    shared = {"mask1": mask1.astype(BF),
              "w1a": np.ascontiguousarray(tonp(fuse_W1)[:, :128].T).astype(BF),
              "w1e": np.ascontiguousarray(tonp(fuse_W1)[:, 128:].T).astype(BF),
              "w2t": np.ascontiguousarray(tonp(fuse_W2).T).astype(BF),
              "w3t": np.ascontiguousarray(tonp(head_W).T).astype(BF),
              "b1": tonp(fuse_b1).reshape(128, 1).astype(np.float32),
              "b2": tonp(fuse_b2).reshape(128, 1).astype(np.float32),
              "b3": tonp(head_b).reshape(2 * TF, 1).astype(np.float32)}
    for k, v in wag.items():
        shared[f"ag_{k}"] = v
    for k, v in weg.items():
        shared[f"eg_{k}"] = v

    in_maps = []
    for c in range(NCORES):
        m = dict(shared)
        xa = x_ag[c * SAG:(c + 1) * SAG]
        ma = m_ag[c * SAG:(c + 1) * SAG]
        for n in range(NCHUNK):
            m[f"xst{n}"] = _xstack(xa[n * NC_AG:(n + 1) * NC_AG],
                                   ma[n * NC_AG:(n + 1) * NC_AG])
        # rotate ego so this core's batches are at columns 0..15
        order = (np.arange(B) + c * BS) % B
        m["xste"] = _xstack(x_eg[order], m_eg[order])
        in_maps.append(m)

    nc = build_nc()
    from concourse.bass_utils import run_bass_kernel_spmd
    import time as _time
    _t0 = _time.time()
    res = run_bass_kernel_spmd(nc, in_maps, list(range(NCORES)))
    _t1 = _time.time()
    # second call reuses the jit-compiled NEFF: wall ~= device exec + dispatch
    res = run_bass_kernel_spmd(nc, in_maps, list(range(NCORES)))
    _t2 = _time.time()
    print("HW exec time:", int((_t2 - _t1) * 1e9), "ns (warm wall, incl dispatch; cold %d ms)" % int((_t1 - _t0) * 1e3))
    global LAST_RESULTS
    LAST_RESULTS = res
    outs = []
    for c in range(NCORES):
        o = np.asarray(res.results[c]["out"], np.float32)   # (60, 1024)
        outs.append(o.T.reshape(BS, NA, TF, 2))
    return np.concatenate(outs, 0).astype(np.float32)


if __name__ == "__main__":
    import reference
    inp = reference.setup_inputs()
    exp = np.asarray(reference.reference(**inp))
    act = kernel(**inp)
    err = np.abs(act - exp).max() / (np.abs(exp).max() + 1e-9)
    print("Relative error:", err)
